# revision 1
# baseline (speedup 1.0000x reference)
"""Causal self-attention on 8 Trainium2 NeuronCores (Bass/Tile).

Problem: x[4, 2048, 1024], w_in[3072, 1024], w_out[1024, 1024], 16 heads.
    qkv = x @ w_in.T ; per-(b,h) causal softmax attention ; out = y @ w_out.T

Sharding (SPMD — one program, per-core input data):
    core c  ->  batch b = c // 2, head-group g = c % 2 (heads 8g .. 8g+7).
    Each core projects q/k/v for its 8 heads of its batch and runs causal
    attention for them.  The pair (2b, 2b+1) AllGathers the two head-group
    halves of yT (chunked per head-pair so it overlaps attention), then each
    core computes the output projection for half of the output features
    (core even: e_out 0..511, odd: 512..1023) over all 2048 tokens of its
    batch.  The host concatenates.

Everything on-chip is kept feature-major ("T" = contraction dim on SBUF
partitions) so no on-device transposes are needed:
    xT [D, S] (host-transposed), qT/kT per head-pair [128, S],
    scoresT [k, q], yT [e, t], outT [e_out, t] (host-transposed back).
Head-pair q/k projections are interleaved with that pair's attention so the
PE stays dense while the ACT engine works through the exps.  Softmax
denominators come from a ones-column appended to V (the AV matmul has
M = 65); normalization is a fast DVE reciprocal on the sum row (moved to
partition 0 by a small DMA — custom DVE ops only work at base partition 0)
+ a K=1 matmul broadcast + one multiply, streamed straight to DRAM.
Matmuls run as float32r (reduced-precision fp32, full PE rate at N >= 256).
"""

import sys

for _p in ("/opt/trn_rl_repo",):
    if _p not in sys.path:
        sys.path.insert(0, _p)

import numpy as np

B, S, D = 4, 2048, 1024
H, HD = 16, 64
N_CORES = 8
HPC = 8            # heads per core
NPAIRS = HPC // 2  # head pairs per core
QC = S // 512      # q-chunks per head
TT = S // 128      # token tiles
DT = D // 128      # feature (d) tiles
EHALF = D // 2     # output features per core

_PROG = None       # cached compiled program


def _build_program():
    import concourse.bass as bass
    from concourse import bacc
    import concourse.tile as tile
    import concourse.mybir as mybir
    from contextlib import ExitStack

    f32 = mybir.dt.float32
    f32r = mybir.dt.float32r
    AF = mybir.ActivationFunctionType
    OP = mybir.AluOpType

    nc = bacc.Bacc("TRN2", target_bir_lowering=False, debug=False,
                   num_devices=N_CORES)

    xT = nc.dram_tensor("xT", [D, S], f32r, kind="ExternalInput").ap()
    wqkT = nc.dram_tensor("wqkT", [D, 2 * HPC * HD], f32r,
                          kind="ExternalInput").ap()
    wvT = nc.dram_tensor("wvT", [D, HPC * HD], f32r, kind="ExternalInput").ap()
    woT = nc.dram_tensor("woT", [D, EHALF], f32r, kind="ExternalInput").ap()
    tri = nc.dram_tensor("tri", [128, 128], f32, kind="ExternalInput").ap()
    outT = nc.dram_tensor("outT", [EHALF, S], f32, kind="ExternalOutput").ap()

    y_loc = nc.dram_tensor("y_loc", [HPC * HD, S], f32r)
    y_gat = [nc.dram_tensor(f"y_gat{i}", [2, 128, S], f32r)
             for i in range(NPAIRS)]

    with tile.TileContext(nc) as tc:
        def mm(out, lhsT, rhs, start, stop):
            nc.tensor.matmul(out, lhsT, rhs, start=start, stop=stop)

        with ExitStack() as perm:
            const_pool = perm.enter_context(tc.tile_pool(name="const", bufs=1))
            v_pool = perm.enter_context(tc.tile_pool(name="vsb", bufs=TT))
            mm_ps = perm.enter_context(
                tc.tile_pool(name="mmps", bufs=2, space="PSUM"))

            tri_sb = const_pool.tile([128, 128], f32, tag="tri")
            nc.sync.dma_start(tri_sb[:], tri[:])
            ones_sb = const_pool.tile([128, 64], f32, tag="ones")
            nc.gpsimd.memset(ones_sb[:], 1.0)
            onesr_sb = const_pool.tile([1, 64], f32r, tag="onesr")
            nc.vector.tensor_copy(onesr_sb[:], ones_sb[0:1, :])

            # v_sb[t]: [128, 8*65] — per head 64 v-columns + a ones column
            v_sb = [v_pool.tile([128, HPC * (HD + 1)], f32r, tag="v",
                                name=f"v{t}") for t in range(TT)]

            with ExitStack() as att_scope:
                qk_pool = att_scope.enter_context(
                    tc.tile_pool(name="qksb", bufs=4))
                xt_pool = att_scope.enter_context(
                    tc.tile_pool(name="xtsb", bufs=DT))
                wqk_pool = att_scope.enter_context(
                    tc.tile_pool(name="wqksb", bufs=2 * DT))
                p_pool = att_scope.enter_context(
                    tc.tile_pool(name="psb", bufs=3))
                n_pool = att_scope.enter_context(
                    tc.tile_pool(name="nsb", bufs=2))
                sc_ps = att_scope.enter_context(
                    tc.tile_pool(name="scps", bufs=2, space="PSUM"))
                y_ps = att_scope.enter_context(
                    tc.tile_pool(name="yps", bufs=2, space="PSUM"))

                xt_sb = [xt_pool.tile([128, S], f32r, tag="xt", name=f"xt{d}")
                         for d in range(DT)]
                for d in range(DT):
                    nc.sync.dma_start(xt_sb[d][:], xT[d * 128:(d + 1) * 128, :])

                # ---- v projection: v[t, e] accumulated over d ----
                with tc.tile_pool(name="wvsb", bufs=DT) as wv_pool:
                    wv_sb = [wv_pool.tile([128, HPC * HD], f32r, tag="wv",
                                          name=f"wv{d}") for d in range(DT)]
                    for d in range(DT):
                        nc.sync.dma_start(wv_sb[d][:],
                                          wvT[d * 128:(d + 1) * 128, :])
                    for t in range(TT):
                        ps = mm_ps.tile([128, 512], f32, tag="mm")
                        for d in range(DT):
                            mm(ps[:], xt_sb[d][:, t * 128:(t + 1) * 128],
                               wv_sb[d][:], start=(d == 0), stop=(d == DT - 1))
                        vdst = v_sb[t][:].rearrange(
                            "p (h e) -> p h e", h=HPC)[:, :, 0:HD]
                        vsrc = ps[:].rearrange("p (h e) -> p h e", h=HPC)
                        nc.vector.tensor_copy(vdst, vsrc)
                        nc.vector.tensor_copy(
                            v_sb[t][:].rearrange(
                                "p (h e) -> p h e", h=HPC)[:, :, HD:HD + 1],
                            ones_sb[:, 0:HPC].unsqueeze(-1))

                # ---- per pair: q/k projection then attention ----
                for i in range(NPAIRS):
                    wqk_sb = [wqk_pool.tile([128, 256], f32r, tag="wqk",
                                            name=f"wqk{i}_{d}")
                              for d in range(DT)]
                    for d in range(DT):
                        nc.sync.dma_start(
                            wqk_sb[d][:, 0:128],
                            wqkT[d * 128:(d + 1) * 128, i * 128:(i + 1) * 128])
                        nc.sync.dma_start(
                            wqk_sb[d][:, 128:256],
                            wqkT[d * 128:(d + 1) * 128,
                                 (NPAIRS + i) * 128:(NPAIRS + i + 1) * 128])
                    q_sb = qk_pool.tile([128, S], f32r, tag="qk", name=f"q{i}")
                    k_sb = qk_pool.tile([128, S], f32r, tag="qk", name=f"k{i}")
                    for which, dest in ((0, q_sb), (1, k_sb)):
                        for qc in range(QC):
                            ps = mm_ps.tile([128, 512], f32, tag="mm")
                            for d in range(DT):
                                mm(ps[:],
                                   wqk_sb[d][:, which * 128:(which + 1) * 128],
                                   xt_sb[d][:, qc * 512:(qc + 1) * 512],
                                   start=(d == 0), stop=(d == DT - 1))
                            nc.vector.tensor_copy(
                                dest[:, qc * 512:(qc + 1) * 512], ps[:])

                    # ---- attention for this pair ----
                    for qc in range(QC):
                        nkt = 4 * qc + 4   # causal: k-tiles 0 .. 4qc+3
                        yps = [y_ps.tile([65, 512], f32, tag="yt",
                                         name=f"yps{i}_{qc}_{h}")
                               for h in range(2)]
                        for kt in range(nkt):
                            j = kt - 4 * qc
                            lo = max(0, j) * 128
                            sc = sc_ps.tile([128, 1024], f32, tag="sc")
                            pt = p_pool.tile([128, 1024], f32r, tag="p")
                            for h in range(2):
                                mm(sc[:, h * 512 + lo:(h + 1) * 512],
                                   k_sb[h * 64:(h + 1) * 64,
                                        kt * 128:(kt + 1) * 128],
                                   q_sb[h * 64:(h + 1) * 64,
                                        qc * 512 + lo:(qc + 1) * 512],
                                   start=True, stop=True)
                            # exp(score / 8) for both heads in one ACT call
                            src = sc[:].rearrange("p (s c) -> p s c", s=2)[
                                :, :, lo:512]
                            dst = pt[:].rearrange("p (s c) -> p s c", s=2)[
                                :, :, lo:512]
                            nc.scalar.activation(dst, src, AF.Exp, scale=0.125)
                            if j >= 0:   # mask the diagonal band
                                for h in range(2):
                                    band = pt[:, h * 512 + lo:
                                              h * 512 + lo + 128]
                                    nc.vector.tensor_mul(band, band, tri_sb[:])
                            for h in range(2):
                                hl = 2 * i + h
                                mm(yps[h][:, lo:512],
                                   v_sb[kt][:, hl * 65:hl * 65 + 65],
                                   pt[:, h * 512 + lo:(h + 1) * 512],
                                   start=(kt == 0), stop=(kt == nkt - 1))
                        # normalize: y[0:64] * (1 / y[64]) and stream to DRAM
                        for h in range(2):
                            ysc = n_pool.tile([65, 512], f32, tag="ysc")
                            nc.vector.tensor_copy(ysc[:], yps[h][:])
                            srow = n_pool.tile([1, 512], f32, tag="srow")
                            nc.sync.dma_start(srow[:], ysc[64:65, :])
                            rcp = n_pool.tile([1, 512], f32, tag="rcp")
                            nc.vector.reciprocal_approx_fast(
                                out=rcp[:], in_=srow[:])
                            rcpr = n_pool.tile([1, 512], f32r, tag="rcpr")
                            nc.vector.tensor_copy(rcpr[:], rcp[:])
                            rb = mm_ps.tile([64, 512], f32, tag="mm")
                            mm(rb[:], onesr_sb[:], rcpr[:],
                               start=True, stop=True)
                            nout = n_pool.tile([64, 512], f32r, tag="nout")
                            nc.vector.tensor_mul(nout[:], ysc[0:64, :], rb[:])
                            nc.sync.dma_start(
                                y_loc[(2 * i + h) * 64:(2 * i + h + 1) * 64,
                                      qc * 512:(qc + 1) * 512], nout[:])

                    # ---- chunked pair AllGather for this head-pair ----
                    nc.gpsimd.collective_compute(
                        "AllGather", OP.bypass,
                        replica_groups=[[0, 1], [2, 3], [4, 5], [6, 7]],
                        ins=[y_loc[i * 128:(i + 1) * 128, :]],
                        outs=[y_gat[i][:]])

            # ---------------- output projection ----------------
            with ExitStack() as oproj:
                wo_pool = oproj.enter_context(
                    tc.tile_pool(name="wosb", bufs=DT))
                yg_pool = oproj.enter_context(
                    tc.tile_pool(name="ygsb", bufs=DT))
                o_pool = oproj.enter_context(tc.tile_pool(name="osb", bufs=2))

                wo_sb = [wo_pool.tile([128, EHALF], f32r, tag="wo",
                                      name=f"wo{d}") for d in range(DT)]
                for d in range(DT):
                    nc.sync.dma_start(wo_sb[d][:],
                                      woT[d * 128:(d + 1) * 128, :])
                yg_sb = [yg_pool.tile([128, S], f32r, tag="yg", name=f"yg{k}")
                         for k in range(DT)]
                for k in range(DT):
                    nc.sync.dma_start(yg_sb[k][:], y_gat[k % NPAIRS][k // NPAIRS])

                for m in range(EHALF // 128):
                    for tch in range(QC):
                        ps = mm_ps.tile([128, 512], f32, tag="mm")
                        for k in range(DT):
                            mm(ps[:], wo_sb[k][:, m * 128:(m + 1) * 128],
                               yg_sb[k][:, tch * 512:(tch + 1) * 512],
                               start=(k == 0), stop=(k == DT - 1))
                        ob = o_pool.tile([128, 512], f32, tag="o")
                        nc.vector.tensor_copy(ob[:], ps[:])
                        nc.sync.dma_start(
                            outT[m * 128:(m + 1) * 128,
                                 tch * 512:(tch + 1) * 512], ob[:])
    nc.finalize()
    return nc


def _prep_inputs(x, w_in, w_out):
    """Build per-core input maps (host-side sharding)."""
    x = np.ascontiguousarray(x, dtype=np.float32)
    w_in = np.ascontiguousarray(w_in, dtype=np.float32)
    w_out = np.ascontiguousarray(w_out, dtype=np.float32)

    tri = np.triu(np.ones((128, 128), dtype=np.float32))  # 1 where k <= q
    in_maps = []
    for c in range(N_CORES):
        b, g = c // 2, c % 2
        heads = [8 * g + h for h in range(HPC)]
        xTb = np.ascontiguousarray(x[b].T)                       # [D, S]
        # wqkT: cols i*128 -> q rows of heads (8g+2i, 8g+2i+1); then k pairs
        qcols, kcols = [], []
        for i in range(NPAIRS):
            hA, hB = heads[2 * i], heads[2 * i + 1]
            qcols.append(w_in[hA * HD:(hA + 1) * HD, :])
            qcols.append(w_in[hB * HD:(hB + 1) * HD, :])
            kcols.append(w_in[D + hA * HD:D + (hA + 1) * HD, :])
            kcols.append(w_in[D + hB * HD:D + (hB + 1) * HD, :])
        wqkT = np.ascontiguousarray(
            np.concatenate(qcols + kcols, axis=0).T)             # [D, 1024]
        wvT = np.ascontiguousarray(np.concatenate(
            [w_in[2 * D + h * HD:2 * D + (h + 1) * HD, :] for h in heads],
            axis=0).T)                                           # [D, 512]
        woT = np.ascontiguousarray(
            w_out[g * EHALF:(g + 1) * EHALF, :].T)               # [D, 512]
        in_maps.append({
            "xT": xTb, "wqkT": wqkT, "wvT": wvT, "woT": woT, "tri": tri,
        })
    return in_maps


def kernel(x, w_in, w_out):
    global _PROG
    from concourse.bass_utils import run_bass_kernel_spmd

    if _PROG is None:
        _PROG = _build_program()
    in_maps = _prep_inputs(x, w_in, w_out)
    res = run_bass_kernel_spmd(_PROG, in_maps, list(range(N_CORES)))

    out = np.empty((B, S, D), dtype=np.float32)
    for c in range(N_CORES):
        b, g = c // 2, c % 2
        out[b, :, g * EHALF:(g + 1) * EHALF] = res.results[c]["outT"].T
    return out



# revision 4
# speedup vs baseline: 1.3751x; 1.3751x over previous
"""Causal self-attention on 8 Trainium2 NeuronCores (Bass/Tile), v2.

Problem: x[4, 2048, 1024], w_in[3072, 1024], w_out[1024, 1024], 16 heads.
    qkv = x @ w_in.T ; per-(b,h) causal softmax attention ; out = y @ w_out.T

Sharding (SPMD, zero collectives):
    core c  ->  batch b = c // 2, head-group g = c % 2 (heads 8g .. 8g+7).
    Each core projects q/k/v for its 8 heads, runs causal attention, then
    computes a PARTIAL output projection over the full 1024 output features
    (contracting only its own 8 heads' y).  The host sums the two partials
    of each batch pair — no on-device collectives, no AllGather round-trip.

Everything on-chip is feature-major ("T" = contraction dim on SBUF
partitions): xT [D, S], qT/kT per head-pair [128, S], scoresT [k, q],
yT [dy, t], outP [e, t].  All matmuls run in bf16 (fp32r streams rows at
half the bf16 rate on TRN2 hardware); PSUM accumulation stays fp32.

Schedule (single linear emission per core, engines pipelined):
  - x arrives in 512-token chunks so the v-projection starts ~3 us in.
  - per head-pair: q/k projection, then causal attention.  The two heads'
    score matmuls contract only 64 dims, so they run concurrently on
    disjoint PE row-groups (auto tile_position).  exp() runs on ACT into
    bf16; the diagonal band is masked by a tri multiply on DVE.
  - softmax denominators ride along as a ones-column appended to V (the
    AV matmul has M = 65); normalization is reciprocal on DVE + a K=1
    broadcast matmul + one multiply.
  - the NEXT pair's q/k projection matmuls and the normalize broadcasts
    are interleaved as "fillers" between attention steps so the PE stays
    dense and warm while ACT works through the exps.
  - output projection accumulates the 4 pair contributions in PSUM
    (K=128 each) and streams bf16 partials to DRAM.
"""

import sys

for _p in ("/opt/trn_rl_repo",):
    if _p not in sys.path:
        sys.path.insert(0, _p)

from collections import deque

import numpy as np

B, S, D = 4, 2048, 1024
H, HD = 16, 64
N_CORES = 8
HPC = 8            # heads per core
NPAIRS = HPC // 2  # head pairs per core
QC = S // 512      # q-chunks per head
TT = S // 128      # token tiles
DT = D // 128      # feature (d) tiles

_PROG = None       # cached compiled program


def _build_program():
    import concourse.bass as bass
    from concourse import bacc
    import concourse.tile as tile
    import concourse.mybir as mybir
    from contextlib import ExitStack

    f32 = mybir.dt.float32
    f32r = mybir.dt.float32r
    bf16 = mybir.dt.bfloat16
    AF = mybir.ActivationFunctionType

    nc = bacc.Bacc("TRN2", target_bir_lowering=False, debug=False,
                   num_devices=N_CORES)

    xT = nc.dram_tensor("xT", [D, S], bf16, kind="ExternalInput").ap()
    wqkT = nc.dram_tensor("wqkT", [D, 2 * HPC * HD], bf16,
                          kind="ExternalInput").ap()
    wvT = nc.dram_tensor("wvT", [D, HPC * HD], bf16, kind="ExternalInput").ap()
    woT = nc.dram_tensor("woT", [HPC * HD, D], bf16, kind="ExternalInput").ap()
    tri = nc.dram_tensor("tri", [128, 128], bf16, kind="ExternalInput").ap()
    outP = nc.dram_tensor("outP", [D, S], bf16, kind="ExternalOutput").ap()

    with tile.TileContext(nc) as tc:
        def mm(out, lhsT, rhs, start, stop):
            nc.tensor.matmul(out, lhsT, rhs, start=start, stop=stop)

        with ExitStack() as perm:
            const_pool = perm.enter_context(tc.tile_pool(name="const", bufs=1))
            xt_pool = perm.enter_context(tc.tile_pool(name="xtsb", bufs=DT))
            wv_pool = perm.enter_context(tc.tile_pool(name="wvsb", bufs=DT))
            wqk_pool = perm.enter_context(
                tc.tile_pool(name="wqksb", bufs=NPAIRS * DT))
            v_pool = perm.enter_context(tc.tile_pool(name="vsb", bufs=TT))
            qk_pool = perm.enter_context(
                tc.tile_pool(name="qksb", bufs=2 * NPAIRS))
            y_pool = perm.enter_context(tc.tile_pool(name="ysb", bufs=NPAIRS))
            wo_pool = perm.enter_context(
                tc.tile_pool(name="wosb", bufs=NPAIRS))
            p_pool = perm.enter_context(tc.tile_pool(name="psb", bufs=3))
            n_pool = perm.enter_context(tc.tile_pool(name="nsb", bufs=4))
            o_pool = perm.enter_context(tc.tile_pool(name="osb", bufs=2))
            sc_ps = perm.enter_context(
                tc.tile_pool(name="scps", bufs=2, space="PSUM"))
            y_ps = perm.enter_context(
                tc.tile_pool(name="yps", bufs=2, space="PSUM"))
            mm_ps = perm.enter_context(
                tc.tile_pool(name="mmps", bufs=2, space="PSUM"))

            # ---- constants ----
            tri_sb = const_pool.tile([128, 128], bf16, tag="tri")
            nc.sync.dma_start(tri_sb[:], tri[:])
            ones_sb = const_pool.tile([128, 64], f32, tag="ones")
            nc.gpsimd.memset(ones_sb[:], 1.0)
            onesr = const_pool.tile([1, 64], f32r, tag="onesr")
            nc.vector.tensor_copy(onesr[:], ones_sb[0:1, :])

            # ---- persistent SBUF tensors ----
            xt_sb = [xt_pool.tile([128, S], bf16, tag="xt", name=f"xt{d}")
                     for d in range(DT)]
            wv_sb = [wv_pool.tile([128, HPC * HD], bf16, tag="wv",
                                  name=f"wv{d}") for d in range(DT)]
            wqk_sb = [[wqk_pool.tile([128, 256], bf16, tag="wqk",
                                     name=f"wqk{i}_{d}") for d in range(DT)]
                      for i in range(NPAIRS)]
            v_sb = [v_pool.tile([128, HPC * (HD + 1)], bf16, tag="v",
                                name=f"v{t}") for t in range(TT)]
            q_sb = [qk_pool.tile([128, S], bf16, tag="qk", name=f"q{i}")
                    for i in range(NPAIRS)]
            k_sb = [qk_pool.tile([128, S], bf16, tag="qk", name=f"k{i}")
                    for i in range(NPAIRS)]
            y_sb = [y_pool.tile([128, S], bf16, tag="y", name=f"y{i}")
                    for i in range(NPAIRS)]
            wo_sb = [wo_pool.tile([128, D], bf16, tag="wo", name=f"wo{i}")
                     for i in range(NPAIRS)]

            # ---- input DMAs, in consumption-priority order ----
            for d in range(DT):
                nc.sync.dma_start(wv_sb[d][:], wvT[d * 128:(d + 1) * 128, :])
            for tc_ in range(QC):
                cs = slice(tc_ * 512, (tc_ + 1) * 512)
                for d in range(DT):
                    nc.sync.dma_start(xt_sb[d][:, cs],
                                      xT[d * 128:(d + 1) * 128, cs])
                if tc_ == 0:
                    for d in range(DT):
                        nc.sync.dma_start(
                            wqk_sb[0][d][:, 0:128],
                            wqkT[d * 128:(d + 1) * 128, 0:128])
                        nc.sync.dma_start(
                            wqk_sb[0][d][:, 128:256],
                            wqkT[d * 128:(d + 1) * 128,
                                 NPAIRS * 128:(NPAIRS + 1) * 128])
            for i in range(1, NPAIRS):
                for d in range(DT):
                    nc.sync.dma_start(
                        wqk_sb[i][d][:, 0:128],
                        wqkT[d * 128:(d + 1) * 128, i * 128:(i + 1) * 128])
                    nc.sync.dma_start(
                        wqk_sb[i][d][:, 128:256],
                        wqkT[d * 128:(d + 1) * 128,
                             (NPAIRS + i) * 128:(NPAIRS + i + 1) * 128])
            for i in range(NPAIRS):
                nc.sync.dma_start(wo_sb[i][:],
                                  woT[i * 128:(i + 1) * 128, :])

            # ---- filler machinery: small PE work units squeezed between
            #      attention steps to keep the PE dense during exp waits ----
            fillers = deque()

            def drain_fillers(n):
                for _ in range(n):
                    if not fillers:
                        return
                    fillers.popleft()()

            def emit_v_group(t):
                ps = mm_ps.tile([128, 512], f32, tag="mm")
                for d in range(DT):
                    mm(ps[:], xt_sb[d][:, t * 128:(t + 1) * 128],
                       wv_sb[d][:], start=(d == 0), stop=(d == DT - 1))
                vdst = v_sb[t][:].rearrange(
                    "p (h e) -> p h e", h=HPC)[:, :, 0:HD]
                vsrc = ps[:].rearrange("p (h e) -> p h e", h=HPC)
                nc.vector.tensor_copy(vdst, vsrc)
                nc.vector.tensor_copy(
                    v_sb[t][:].rearrange(
                        "p (h e) -> p h e", h=HPC)[:, :, HD:HD + 1],
                    ones_sb[:, 0:HPC].unsqueeze(-1))

            def emit_qk_group(i, which, qc):
                """One q or k projection chunk: 8 matmuls + cast."""
                dest = q_sb[i] if which == 0 else k_sb[i]
                ps = mm_ps.tile([128, 512], f32, tag="mm")
                for d in range(DT):
                    mm(ps[:], wqk_sb[i][d][:, which * 128:(which + 1) * 128],
                       xt_sb[d][:, qc * 512:(qc + 1) * 512],
                       start=(d == 0), stop=(d == DT - 1))
                nc.vector.tensor_copy(dest[:, qc * 512:(qc + 1) * 512], ps[:])

            def qk_units(i):
                """Split pair i's q/k projection into 8 filler closures."""
                return [
                    (lambda i=i, w=w, qc=qc: emit_qk_group(i, w, qc))
                    for qc in range(QC) for w in (0, 1)
                ]

            def emit_norm_head(i, qc, h, ysc, rcpr):
                """Broadcast 1/den over 64 partitions and scale y."""
                ps_rb = mm_ps.tile([64, 512], f32, tag="mm")
                mm(ps_rb[:], onesr[:], rcpr[:], start=True, stop=True)
                if h == 0:
                    nc.vector.tensor_mul(
                        y_sb[i][0:64, qc * 512:(qc + 1) * 512],
                        ysc[0:64, :], ps_rb[:])
                else:
                    yt = n_pool.tile([64, 512], bf16, tag="yt")
                    nc.vector.tensor_mul(yt[:], ysc[0:64, :], ps_rb[:])
                    nc.sync.dma_start(
                        y_sb[i][64:128, qc * 512:(qc + 1) * 512], yt[:])

            def emit_attention(i):
                for qc in range(QC):
                    nkt = 4 * qc + 4   # causal: k-tiles 0 .. 4qc+3
                    yps = [y_ps.tile([65, 512], f32, tag="yp",
                                     name=f"yps{i}_{qc}_{h}")
                           for h in range(2)]
                    for kt in range(nkt):
                        j = kt - 4 * qc
                        lo = max(0, j) * 128
                        sc = sc_ps.tile([128, 1024], f32, tag="sc")
                        pt = p_pool.tile([128, 1024], bf16, tag="p")
                        for h in range(2):
                            mm(sc[:, h * 512 + lo:(h + 1) * 512],
                               k_sb[i][h * 64:(h + 1) * 64,
                                       kt * 128:(kt + 1) * 128],
                               q_sb[i][h * 64:(h + 1) * 64,
                                       qc * 512 + lo:(qc + 1) * 512],
                               start=True, stop=True)
                        drain_fillers(1)
                        # exp(score / 8) for both heads in one ACT call
                        src = sc[:].rearrange("p (s c) -> p s c", s=2)[
                            :, :, lo:512]
                        dst = pt[:].rearrange("p (s c) -> p s c", s=2)[
                            :, :, lo:512]
                        nc.scalar.activation(dst, src, AF.Exp, scale=0.125)
                        if j >= 0:   # mask the diagonal band
                            for h in range(2):
                                band = pt[:, h * 512 + lo:h * 512 + lo + 128]
                                nc.vector.tensor_mul(band, band, tri_sb[:])
                        drain_fillers(1)
                        for h in range(2):
                            hl = 2 * i + h
                            mm(yps[h][:, lo:512],
                               v_sb[kt][:, hl * 65:hl * 65 + 65],
                               pt[:, h * 512 + lo:(h + 1) * 512],
                               start=(kt == 0), stop=(kt == nkt - 1))
                    # end of q-chunk: free PSUM fast, defer the normalize
                    for h in range(2):
                        ysc = n_pool.tile([65, 512], f32, tag="ysc")
                        nc.vector.tensor_copy(ysc[:], yps[h][:])
                        srow = n_pool.tile([1, 512], f32, tag="srow")
                        nc.sync.dma_start(srow[:], ysc[64:65, :])
                        rcp = n_pool.tile([1, 512], f32, tag="rcp")
                        nc.vector.reciprocal_approx_fast(
                            out=rcp[:], in_=srow[:])
                        rcpr = n_pool.tile([1, 512], f32r, tag="rcpr")
                        nc.vector.tensor_copy(rcpr[:], rcp[:])
                        fillers.append(
                            lambda i=i, qc=qc, h=h, ysc=ysc, rcpr=rcpr:
                            emit_norm_head(i, qc, h, ysc, rcpr))

            # ---------------- phase A: v-proj + pair-0 qk + pair-0 att ----
            for tc_ in range(QC):
                for t in range(4 * tc_, 4 * tc_ + 4):
                    emit_v_group(t)
                emit_qk_group(0, 0, tc_)
                emit_qk_group(0, 1, tc_)
                if tc_ == 0:
                    fillers.extend(qk_units(1))
                emit_attention_qc = tc_  # attention for pair 0, chunk tc_
                # (emitted inline below to keep v/qk/att interleaved)
                if tc_ == 0:
                    pass
            # pair 0 attention runs after its projections; later pairs
            # get their projections via fillers during earlier attention.
            emit_attention(0)
            for i in range(1, NPAIRS):
                if i + 1 < NPAIRS:
                    fillers.extend(qk_units(i + 1))
                emit_attention(i)

            # ---------------- partial output projection ----------------
            drain_fillers(len(fillers))
            for m in range(DT):
                for tch in range(QC):
                    ps = mm_ps.tile([128, 512], f32, tag="mm")
                    for i in range(NPAIRS):
                        mm(ps[:], wo_sb[i][:, m * 128:(m + 1) * 128],
                           y_sb[i][:, tch * 512:(tch + 1) * 512],
                           start=(i == 0), stop=(i == NPAIRS - 1))
                    ob = o_pool.tile([128, 512], bf16, tag="o")
                    nc.vector.tensor_copy(ob[:], ps[:])
                    nc.sync.dma_start(
                        outP[m * 128:(m + 1) * 128,
                             tch * 512:(tch + 1) * 512], ob[:])
    nc.finalize()
    return nc


def _prep_inputs(x, w_in, w_out):
    """Build per-core input maps (host-side sharding), bf16."""
    import ml_dtypes
    bf16 = ml_dtypes.bfloat16

    x = np.ascontiguousarray(x, dtype=np.float32)
    w_in = np.ascontiguousarray(w_in, dtype=np.float32)
    w_out = np.ascontiguousarray(w_out, dtype=np.float32)

    tri = np.triu(np.ones((128, 128), dtype=np.float32))  # 1 where k <= q
    in_maps = []
    for c in range(N_CORES):
        b, g = c // 2, c % 2
        heads = [8 * g + h for h in range(HPC)]
        xTb = np.ascontiguousarray(x[b].T.astype(bf16))          # [D, S]
        # wqkT: cols i*128 -> q rows of heads (8g+2i, 8g+2i+1); then k pairs
        qcols, kcols, wocols = [], [], []
        for i in range(NPAIRS):
            hA, hB = heads[2 * i], heads[2 * i + 1]
            qcols.append(w_in[hA * HD:(hA + 1) * HD, :])
            qcols.append(w_in[hB * HD:(hB + 1) * HD, :])
            kcols.append(w_in[D + hA * HD:D + (hA + 1) * HD, :])
            kcols.append(w_in[D + hB * HD:D + (hB + 1) * HD, :])
            wocols.append(w_out[:, hA * HD:(hA + 1) * HD])
            wocols.append(w_out[:, hB * HD:(hB + 1) * HD])
        wqkT = np.ascontiguousarray(
            np.concatenate(qcols + kcols, axis=0).T.astype(bf16))  # [D, 1024]
        wvT = np.ascontiguousarray(np.concatenate(
            [w_in[2 * D + h * HD:2 * D + (h + 1) * HD, :] for h in heads],
            axis=0).T.astype(bf16))                               # [D, 512]
        # woT rows: pair i -> y rows (hA 64 | hB 64); cols: all 1024 e
        woT = np.ascontiguousarray(
            np.concatenate(wocols, axis=1).T.astype(bf16))        # [512, 1024]
        in_maps.append({
            "xT": xTb, "wqkT": wqkT, "wvT": wvT, "woT": woT,
            "tri": tri.astype(bf16),
        })
    return in_maps


def kernel(x, w_in, w_out):
    global _PROG
    from concourse.bass_utils import run_bass_kernel_spmd

    if _PROG is None:
        _PROG = _build_program()
    in_maps = _prep_inputs(x, w_in, w_out)
    res = run_bass_kernel_spmd(_PROG, in_maps, list(range(N_CORES)))

    out = np.empty((B, S, D), dtype=np.float32)
    for b in range(B):
        pe = res.results[2 * b]["outP"].astype(np.float32)
        po = res.results[2 * b + 1]["outP"].astype(np.float32)
        out[b] = (pe + po).T
    return out


# revision 5
# speedup vs baseline: 1.4724x; 1.0708x over previous
"""Causal self-attention on 8 Trainium2 NeuronCores (Bass/Tile), v3.

Problem: x[4, 2048, 1024], w_in[3072, 1024], w_out[1024, 1024], 16 heads.
    qkv = x @ w_in.T ; per-(b,h) causal softmax attention ; out = y @ w_out.T

Sharding (SPMD, zero collectives):
    core c  ->  batch b = c // 2, head-group g = c % 2 (heads 8g .. 8g+7).
    Each core projects q/k/v for its 8 heads, runs causal attention, then
    computes a PARTIAL output projection over the full 1024 output features
    (contracting only its own 8 heads' y).  The host sums the two partials
    of each batch pair — no on-device collectives, no AllGather round-trip.

Everything on-chip is feature-major ("T" = contraction dim on SBUF
partitions): xT [D, S], qT/kT per head-pair [128, S], scoresT [k, q],
yT [dy, t], outP [e, t].  All matmuls run in bf16 (fp32r streams rows at
half the bf16 rate on TRN2 hardware); PSUM accumulation stays fp32.

Schedule (single linear emission per core, engines pipelined):
  - bulk input DMAs ride the idle GpSimd queue; the critical first tiles
    (wv, x chunk 0, wqk pair 0) go on SP so the PE starts ~10 us in.
  - pair-0 attention is interleaved with the v-projection chunks.
  - the two heads' score matmuls contract only 64 dims each, so they run
    concurrently on disjoint PE row-groups (auto tile_position).  exp()
    runs on ACT into bf16; the diagonal band is masked by a tri multiply.
  - softmax denominators ride along as a ones-column appended to V (the
    AV matmul has M = 65); normalization is reciprocal on DVE + a K=1
    broadcast matmul + one multiply.
  - "fillers" (the next pair's q/k projection, deferred normalizations,
    and — during the last pair — most of the output projection) are
    drained between attention steps at a fixed cadence so the PE stays
    dense and warm while ACT works through the exps.
"""

import sys

for _p in ("/opt/trn_rl_repo",):
    if _p not in sys.path:
        sys.path.insert(0, _p)

from collections import deque

import numpy as np

B, S, D = 4, 2048, 1024
H, HD = 16, 64
N_CORES = 8
HPC = 8            # heads per core
NPAIRS = HPC // 2  # head pairs per core
QC = S // 512      # q-chunks per head
TT = S // 128      # token tiles
DT = D // 128      # feature (d) tiles

_PROG = None       # cached compiled program


def _build_program():
    import concourse.bass as bass
    from concourse import bacc
    import concourse.tile as tile
    import concourse.mybir as mybir
    from contextlib import ExitStack

    f32 = mybir.dt.float32
    bf16 = mybir.dt.bfloat16
    AF = mybir.ActivationFunctionType

    nc = bacc.Bacc("TRN2", target_bir_lowering=False, debug=False,
                   num_devices=N_CORES)

    xT = nc.dram_tensor("xT", [D, S], bf16, kind="ExternalInput").ap()
    wqkT = nc.dram_tensor("wqkT", [D, NPAIRS * 256], bf16,
                          kind="ExternalInput").ap()
    wvT = nc.dram_tensor("wvT", [D, HPC * HD], bf16, kind="ExternalInput").ap()
    woT = nc.dram_tensor("woT", [HPC * HD, D], bf16, kind="ExternalInput").ap()
    tri = nc.dram_tensor("tri", [128, 128], bf16, kind="ExternalInput").ap()
    outP = nc.dram_tensor("outP", [D, S], bf16, kind="ExternalOutput").ap()

    with tile.TileContext(nc) as tc:
        def mm(out, lhsT, rhs, start, stop):
            nc.tensor.matmul(out, lhsT, rhs, start=start, stop=stop)

        with ExitStack() as perm:
            const_pool = perm.enter_context(tc.tile_pool(name="const", bufs=1))
            xt_pool = perm.enter_context(tc.tile_pool(name="xtsb", bufs=DT))
            wv_pool = perm.enter_context(tc.tile_pool(name="wvsb", bufs=DT))
            wqk_pool = perm.enter_context(
                tc.tile_pool(name="wqksb", bufs=NPAIRS * DT))
            v_pool = perm.enter_context(tc.tile_pool(name="vsb", bufs=TT))
            qk_pool = perm.enter_context(
                tc.tile_pool(name="qksb", bufs=2 * NPAIRS))
            y_pool = perm.enter_context(tc.tile_pool(name="ysb", bufs=NPAIRS))
            wo_pool = perm.enter_context(
                tc.tile_pool(name="wosb", bufs=NPAIRS))
            p_pool = perm.enter_context(tc.tile_pool(name="psb", bufs=3))
            n_pool = perm.enter_context(tc.tile_pool(name="nsb", bufs=4))
            o_pool = perm.enter_context(tc.tile_pool(name="osb", bufs=2))
            sc_ps = perm.enter_context(
                tc.tile_pool(name="scps", bufs=2, space="PSUM"))
            y_ps = perm.enter_context(
                tc.tile_pool(name="yps", bufs=2, space="PSUM"))
            mm_ps = perm.enter_context(
                tc.tile_pool(name="mmps", bufs=2, space="PSUM"))

            # ---- constants ----
            tri_sb = const_pool.tile([128, 128], bf16, tag="tri")
            nc.sync.dma_start(tri_sb[:], tri[:])
            ones_sb = const_pool.tile([128, 64], f32, tag="ones")
            nc.gpsimd.memset(ones_sb[:], 1.0)
            onesb = const_pool.tile([1, 64], bf16, tag="onesb")
            nc.vector.tensor_copy(onesb[:], ones_sb[0:1, :])

            # ---- persistent SBUF tensors ----
            xt_sb = [xt_pool.tile([128, S], bf16, tag="xt", name=f"xt{d}")
                     for d in range(DT)]
            wv_sb = [wv_pool.tile([128, HPC * HD], bf16, tag="wv",
                                  name=f"wv{d}") for d in range(DT)]
            wqk_sb = [[wqk_pool.tile([128, 256], bf16, tag="wqk",
                                     name=f"wqk{i}_{d}") for d in range(DT)]
                      for i in range(NPAIRS)]
            v_sb = [v_pool.tile([128, HPC * (HD + 1)], bf16, tag="v",
                                name=f"v{t}") for t in range(TT)]
            q_sb = [qk_pool.tile([128, S], bf16, tag="qk", name=f"q{i}")
                    for i in range(NPAIRS)]
            k_sb = [qk_pool.tile([128, S], bf16, tag="qk", name=f"k{i}")
                    for i in range(NPAIRS)]
            y_sb = [y_pool.tile([128, S], bf16, tag="y", name=f"y{i}")
                    for i in range(NPAIRS)]
            wo_sb = [wo_pool.tile([128, D], bf16, tag="wo", name=f"wo{i}")
                     for i in range(NPAIRS)]

            # ---- input DMAs: critical path on SP, bulk on idle GpSimd ----
            for d in range(DT):
                nc.sync.dma_start(wv_sb[d][:], wvT[d * 128:(d + 1) * 128, :])
            for d in range(DT):
                nc.sync.dma_start(xt_sb[d][:, 0:512],
                                  xT[d * 128:(d + 1) * 128, 0:512])
            for d in range(DT):
                nc.sync.dma_start(wqk_sb[0][d][:],
                                  wqkT[d * 128:(d + 1) * 128, 0:256])
            for d in range(DT):
                nc.gpsimd.dma_start(xt_sb[d][:, 512:S],
                                    xT[d * 128:(d + 1) * 128, 512:S])
            for i in range(1, NPAIRS):
                for d in range(DT):
                    nc.gpsimd.dma_start(
                        wqk_sb[i][d][:],
                        wqkT[d * 128:(d + 1) * 128, i * 256:(i + 1) * 256])
            for i in range(NPAIRS):
                nc.gpsimd.dma_start(wo_sb[i][:],
                                    woT[i * 128:(i + 1) * 128, :])

            # ---- filler machinery: small PE work units squeezed between
            #      attention steps to keep the PE dense during exp waits ----
            fillers = deque()
            step_ctr = [0]

            def drain_one():
                if fillers:
                    fillers.popleft()()

            def emit_v_group(t):
                ps = mm_ps.tile([128, 512], f32, tag="mm")
                for d in range(DT):
                    mm(ps[:], xt_sb[d][:, t * 128:(t + 1) * 128],
                       wv_sb[d][:], start=(d == 0), stop=(d == DT - 1))
                vdst = v_sb[t][:].rearrange(
                    "p (h e) -> p h e", h=HPC)[:, :, 0:HD]
                vsrc = ps[:].rearrange("p (h e) -> p h e", h=HPC)
                nc.vector.tensor_copy(vdst, vsrc)
                nc.vector.tensor_copy(
                    v_sb[t][:].rearrange(
                        "p (h e) -> p h e", h=HPC)[:, :, HD:HD + 1],
                    ones_sb[:, 0:HPC].unsqueeze(-1))

            def emit_qk_group(i, which, qc):
                """One q or k projection chunk: 8 matmuls + cast."""
                dest = q_sb[i] if which == 0 else k_sb[i]
                ps = mm_ps.tile([128, 512], f32, tag="mm")
                for d in range(DT):
                    mm(ps[:], wqk_sb[i][d][:, which * 128:(which + 1) * 128],
                       xt_sb[d][:, qc * 512:(qc + 1) * 512],
                       start=(d == 0), stop=(d == DT - 1))
                nc.vector.tensor_copy(dest[:, qc * 512:(qc + 1) * 512], ps[:])

            def qk_units(i):
                return [
                    (lambda i=i, w=w, qc=qc: emit_qk_group(i, w, qc))
                    for qc in range(QC) for w in (0, 1)
                ]

            def emit_norm_head(i, qc, h, ysc, rcpb):
                """Broadcast 1/den over 64 partitions and scale y."""
                ps_rb = mm_ps.tile([64, 512], f32, tag="mm")
                mm(ps_rb[:], onesb[:], rcpb[:], start=True, stop=True)
                if h == 0:
                    nc.vector.tensor_mul(
                        y_sb[i][0:64, qc * 512:(qc + 1) * 512],
                        ysc[0:64, :], ps_rb[:])
                else:
                    yt = n_pool.tile([64, 512], bf16, tag="yt")
                    nc.vector.tensor_mul(yt[:], ysc[0:64, :], ps_rb[:])
                    nc.sync.dma_start(
                        y_sb[i][64:128, qc * 512:(qc + 1) * 512], yt[:])

            def emit_out_group(m, tch):
                """Partial output projection for one (e-tile, t-chunk)."""
                ps = mm_ps.tile([128, 512], f32, tag="mm")
                for i in range(NPAIRS):
                    mm(ps[:], wo_sb[i][:, m * 128:(m + 1) * 128],
                       y_sb[i][:, tch * 512:(tch + 1) * 512],
                       start=(i == 0), stop=(i == NPAIRS - 1))
                ob = o_pool.tile([128, 512], bf16, tag="o")
                nc.vector.tensor_copy(ob[:], ps[:])
                nc.sync.dma_start(
                    outP[m * 128:(m + 1) * 128,
                         tch * 512:(tch + 1) * 512], ob[:])

            def emit_attention_qc(i, qc, cadence):
                nkt = 4 * qc + 4   # causal: k-tiles 0 .. 4qc+3
                yps = [y_ps.tile([65, 512], f32, tag="yp",
                                 name=f"yps{i}_{qc}_{h}")
                       for h in range(2)]
                for kt in range(nkt):
                    j = kt - 4 * qc
                    lo = max(0, j) * 128
                    sc = sc_ps.tile([128, 1024], f32, tag="sc")
                    pt = p_pool.tile([128, 1024], bf16, tag="p")
                    for h in range(2):
                        mm(sc[:, h * 512 + lo:(h + 1) * 512],
                           k_sb[i][h * 64:(h + 1) * 64,
                                   kt * 128:(kt + 1) * 128],
                           q_sb[i][h * 64:(h + 1) * 64,
                                   qc * 512 + lo:(qc + 1) * 512],
                           start=True, stop=True)
                    # exp(score / 8) for both heads in one ACT call
                    src = sc[:].rearrange("p (s c) -> p s c", s=2)[
                        :, :, lo:512]
                    dst = pt[:].rearrange("p (s c) -> p s c", s=2)[
                        :, :, lo:512]
                    nc.scalar.activation(dst, src, AF.Exp, scale=0.125)
                    if j >= 0:   # mask the diagonal band
                        for h in range(2):
                            band = pt[:, h * 512 + lo:h * 512 + lo + 128]
                            nc.vector.tensor_mul(band, band, tri_sb[:])
                    step_ctr[0] += 1
                    if step_ctr[0] % cadence == 0:
                        drain_one()
                    for h in range(2):
                        hl = 2 * i + h
                        mm(yps[h][:, lo:512],
                           v_sb[kt][:, hl * 65:hl * 65 + 65],
                           pt[:, h * 512 + lo:(h + 1) * 512],
                           start=(kt == 0), stop=(kt == nkt - 1))
                # end of q-chunk: free PSUM fast, defer the normalize
                for h in range(2):
                    ysc = n_pool.tile([65, 512], f32, tag="ysc")
                    nc.vector.tensor_copy(ysc[:], yps[h][:])
                    srow = n_pool.tile([1, 512], f32, tag="srow")
                    nc.sync.dma_start(srow[:], ysc[64:65, :])
                    rcp = n_pool.tile([1, 512], f32, tag="rcp")
                    nc.vector.reciprocal_approx_fast(out=rcp[:], in_=srow[:])
                    rcpb = n_pool.tile([1, 512], bf16, tag="rcpb")
                    nc.vector.tensor_copy(rcpb[:], rcp[:])
                    fillers.append(
                        lambda i=i, qc=qc, h=h, ysc=ysc, rcpb=rcpb:
                        emit_norm_head(i, qc, h, ysc, rcpb))

            # ------- phase A: v-proj + pair-0 q/k + pair-0 attention -------
            fillers.extend(qk_units(1))
            for tc_ in range(QC):
                for t in range(4 * tc_, 4 * tc_ + 4):
                    emit_v_group(t)
                emit_qk_group(0, 0, tc_)
                emit_qk_group(0, 1, tc_)
                emit_attention_qc(0, tc_, cadence=2)

            # ------- pairs 1..3, with out-proj folded into pair 3 -------
            for i in range(1, NPAIRS):
                if i + 1 < NPAIRS:
                    fillers.extend(qk_units(i + 1))
                for qc in range(QC):
                    if i == NPAIRS - 1 and qc >= 1:
                        fillers.extend([
                            (lambda m=m, tch=qc - 1: emit_out_group(m, tch))
                            for m in range(DT)])
                    emit_attention_qc(i, qc,
                                      cadence=1 if i == NPAIRS - 1 else 2)

            # ------- tail: leftover fillers + last out-proj chunk -------
            while fillers:
                drain_one()
            for m in range(DT):
                emit_out_group(m, QC - 1)
    nc.finalize()
    return nc


def _prep_inputs(x, w_in, w_out):
    """Build per-core input maps (host-side sharding), bf16."""
    import ml_dtypes
    bf16 = ml_dtypes.bfloat16

    x = np.ascontiguousarray(x, dtype=np.float32)
    w_in = np.ascontiguousarray(w_in, dtype=np.float32)
    w_out = np.ascontiguousarray(w_out, dtype=np.float32)

    tri = np.triu(np.ones((128, 128), dtype=np.float32))  # 1 where k <= q
    in_maps = []
    for c in range(N_CORES):
        b, g = c // 2, c % 2
        heads = [8 * g + h for h in range(HPC)]
        xTb = np.ascontiguousarray(x[b].T.astype(bf16))          # [D, S]
        # wqkT block i (256 cols): [q_hA | q_hB | k_hA | k_hB]
        blocks, wocols = [], []
        for i in range(NPAIRS):
            hA, hB = heads[2 * i], heads[2 * i + 1]
            blocks += [w_in[hA * HD:(hA + 1) * HD, :],
                       w_in[hB * HD:(hB + 1) * HD, :],
                       w_in[D + hA * HD:D + (hA + 1) * HD, :],
                       w_in[D + hB * HD:D + (hB + 1) * HD, :]]
            wocols.append(w_out[:, hA * HD:(hA + 1) * HD])
            wocols.append(w_out[:, hB * HD:(hB + 1) * HD])
        wqkT = np.ascontiguousarray(
            np.concatenate(blocks, axis=0).T.astype(bf16))       # [D, 1024]
        wvT = np.ascontiguousarray(np.concatenate(
            [w_in[2 * D + h * HD:2 * D + (h + 1) * HD, :] for h in heads],
            axis=0).T.astype(bf16))                               # [D, 512]
        # woT rows: pair i -> y rows (hA 64 | hB 64); cols: all 1024 e
        woT = np.ascontiguousarray(
            np.concatenate(wocols, axis=1).T.astype(bf16))        # [512, 1024]
        in_maps.append({
            "xT": xTb, "wqkT": wqkT, "wvT": wvT, "woT": woT,
            "tri": tri.astype(bf16),
        })
    return in_maps


def kernel(x, w_in, w_out):
    global _PROG
    from concourse.bass_utils import run_bass_kernel_spmd

    if _PROG is None:
        _PROG = _build_program()
    in_maps = _prep_inputs(x, w_in, w_out)
    res = run_bass_kernel_spmd(_PROG, in_maps, list(range(N_CORES)))

    out = np.empty((B, S, D), dtype=np.float32)
    for b in range(B):
        pe = res.results[2 * b]["outP"].astype(np.float32)
        po = res.results[2 * b + 1]["outP"].astype(np.float32)
        out[b] = (pe + po).T
    return out
